# revision 4
# baseline (speedup 1.0000x reference)
"""Local cross-attention (kNN gather) Trainium2 Bass kernel — v2.

Data-parallel over the 40000 query points across 8 NeuronCores.

Per core:
  Phase A: project all keys to a bf16 KV table in DRAM scratch
           (row n = [K-row(n) bf16 x128 | V-row(n) bf16 x128], 512B).
           Matmuls run key-major (lhsT = keysT chunk, rhs = [Wk|Wv_perm])
           so no PE transposes are needed; ACT copies PSUM->SBUF bf16.
  Phase B: per tile of 128 queries: 32 indirect-DMA row gathers (512B
           rows), scores + softmax + weighted sum on DVE in bf16 2x mode
           with halving-tree reductions, output projection on PE.

Bias algebra (exact):
  bk drops out (softmax is invariant to a per-(q,h) score shift).
  bv folds into the output bias: bo_eff = bv @ Wo + bo (host-side).
  bq is added into the Q projection via a ones-row accumulate matmul.

V layout: host permutes Wv columns hd-major (j = hd*H + h) so the
softmax-weight broadcast in the V aggregation has unit innermost stride
(keeps DVE in 2x mode); Wo rows are permuted to match.
"""

import numpy as np
import ml_dtypes

N1, N2, D, H, K = 40000, 60000, 128, 8, 32
HD = D // H
SCALE = HD ** -0.5
NCORES = 8
N1C = N1 // NCORES          # 5000 queries per core
QT = 128                    # queries per tile
N1P = 5120                  # padded queries per core -> 40 tiles
NT = N1P // QT
N2P = 60416                 # padded key count = 472*128
NSUB = N2P // 128           # phase-A subtiles of 128 keys
E = 2 * D                   # interleaved KV row length (256 bf16 = 512B)

_PROG = None


def _build():
    import concourse.bass as bass
    import concourse.tile as tile
    from concourse import bacc, mybir
    from contextlib import ExitStack

    f32 = mybir.dt.float32
    bf16 = mybir.dt.bfloat16
    i32 = mybir.dt.int32
    AX = mybir.AxisListType
    OP = mybir.AluOpType
    AF = mybir.ActivationFunctionType

    nc = bacc.Bacc("TRN2", target_bir_lowering=False, debug=False,
                   enable_asserts=False, num_devices=1)

    qT = nc.dram_tensor("qT", [D, N1P], bf16, kind="ExternalInput").ap()
    keysT = nc.dram_tensor("keysT", [D, N2P], bf16, kind="ExternalInput").ap()
    knn = nc.dram_tensor("knn", [N1P, K], i32, kind="ExternalInput").ap()
    wq = nc.dram_tensor("wq", [D, D], bf16, kind="ExternalInput").ap()
    wkv = nc.dram_tensor("wkv", [D, E], bf16, kind="ExternalInput").ap()
    wo = nc.dram_tensor("wo", [D, D], bf16, kind="ExternalInput").ap()
    bq = nc.dram_tensor("bq", [1, D], bf16, kind="ExternalInput").ap()
    bo = nc.dram_tensor("bo", [1, D], bf16, kind="ExternalInput").ap()
    outD = nc.dram_tensor("outD", [N1P, D], f32, kind="ExternalOutput").ap()
    table = nc.dram_tensor("kv_table", [N2P, E], bf16, kind="Internal").ap()

    with tile.TileContext(nc) as tc:
        with ExitStack() as cst:
            cp = cst.enter_context(tc.tile_pool(name="const", bufs=1))
            from concourse.masks import make_identity
            ident = cp.tile([128, 128], bf16)
            make_identity(nc, ident[:])
            wq_s = cp.tile([D, D], bf16, tag="wq")
            wkv_s = cp.tile([D, E], bf16, tag="wkv")
            wo_s = cp.tile([D, D], bf16, tag="wo")
            bq_s = cp.tile([1, D], bf16, tag="bq")
            bo_s = cp.tile([1, D], bf16, tag="bo")
            ones_s = cp.tile([1, QT], bf16, tag="ones")
            nc.vector.memset(ones_s[:], 1.0)
            for sb, dr in ((wq_s, wq), (wkv_s, wkv), (wo_s, wo),
                           (bq_s, bq), (bo_s, bo)):
                nc.sync.dma_start(sb[:], dr)
            qT_s = cp.tile([D, N1P], bf16, tag="qTs")
            nc.sync.dma_start(qT_s[:], qT)

            # ---------------- Phase A: build KV table ----------------
            with ExitStack() as ast:
                ap_ = ast.enter_context(tc.tile_pool(name="pa_sb", bufs=3))
                kvp_ = ast.enter_context(tc.tile_pool(name="pa_kv", bufs=3))
                psA = ast.enter_context(
                    tc.tile_pool(name="pa_ps", bufs=4, space="PSUM"))
                for c in range(NSUB // 4):
                    kc = ap_.tile([128, 4 * 128], bf16, tag="kc")
                    nc.sync.dma_start(kc[:], keysT[:, bass.ts(c, 512)])
                    kvs = kvp_.tile([128, 4 * E], bf16, tag="kvs")
                    for t in range(4):
                        psKV = psA.tile([128, E], f32, tag="psKV")
                        nc.tensor.matmul(psKV[:],
                                         lhsT=kc[:, bass.ts(t, 128)],
                                         rhs=wkv_s[:], start=True, stop=True)
                        nc.scalar.activation(kvs[:, bass.ts(t, E)], psKV[:],
                                             AF.Copy)
                    nc.sync.dma_start(
                        table[c * 512:(c + 1) * 512, :]
                        .rearrange("(t p) e -> p t e", p=128),
                        kvs[:].rearrange("p (t e) -> p t e", e=E))

            # Phase A writes kv_table in DRAM; Tile does not track DRAM
            # hazards, so fence before phase B gathers from it.
            tc.strict_bb_all_engine_barrier()

            # ---------------- Phase B: gather + attention ----------------
            with ExitStack() as bst:
                ixp = bst.enter_context(tc.tile_pool(name="pb_ix", bufs=3))
                kvp = bst.enter_context(tc.tile_pool(name="pb_kv", bufs=3))
                ppp = bst.enter_context(tc.tile_pool(name="pb_prod", bufs=2))
                trp = bst.enter_context(tc.tile_pool(name="pb_tree", bufs=2))
                ssp = bst.enter_context(tc.tile_pool(name="pb_small", bufs=3))
                psp = bst.enter_context(
                    tc.tile_pool(name="pb_ps", bufs=2, space="PSUM"))
                for i in range(NT):
                    idx = ixp.tile([128, K], i32, tag="idx")
                    nc.sync.dma_start(idx[:], knn[bass.ts(i, 128), :])
                    kv = kvp.tile([128, K * E], bf16, tag="kv")
                    kv3 = kv[:].rearrange("p (k e) -> p k e", e=E)
                    # one 512B row per partition per indirect DMA
                    for k in range(K):
                        nc.gpsimd.indirect_dma_start(
                            out=kv[:, k * E:(k + 1) * E],
                            out_offset=None, in_=table,
                            in_offset=bass.IndirectOffsetOnAxis(
                                ap=idx[:, k:k + 1], axis=0))

                    # Q projection -> [q, d] with bq, scaled by SCALE
                    psQ = psp.tile([128, D], f32, tag="psQ")
                    nc.tensor.matmul(psQ[:], lhsT=qT_s[:, bass.ts(i, QT)],
                                     rhs=wq_s[:], start=True, stop=False)
                    nc.tensor.matmul(psQ[:], lhsT=ones_s[:], rhs=bq_s[:],
                                     start=False, stop=True)
                    qs = ssp.tile([128, D], bf16, tag="qs")
                    nc.scalar.activation(qs[:], psQ[:], AF.Copy, scale=SCALE)

                    # scores: prod[q, k, d] = K_g[q,k,d] * qs[q,d]  (2x)
                    prod = ppp.tile([128, K * D], bf16, tag="prod")
                    nc.vector.tensor_tensor(
                        out=prod[:].rearrange("p (k d) -> p k d", d=D),
                        in0=kv3[:, :, 0:D],
                        in1=qs[:].unsqueeze(1).broadcast_to([128, K, D]),
                        op=OP.mult)
                    # halving-tree reduce over d within each head:
                    # prod layout (k, h, hd16) -> sc[q, (k,h)]
                    t1 = trp.tile([128, K * H * 8], bf16, tag="t1")
                    nc.vector.tensor_tensor(
                        out=t1[:].rearrange("p (s d) -> p s d", d=8),
                        in0=prod[:].rearrange("p (s d) -> p s d", d=16)[:, :, 0:8],
                        in1=prod[:].rearrange("p (s d) -> p s d", d=16)[:, :, 8:16],
                        op=OP.add)
                    t2 = trp.tile([128, K * H * 4], bf16, tag="t2")
                    nc.vector.tensor_tensor(
                        out=t2[:].rearrange("p (s d) -> p s d", d=4),
                        in0=t1[:].rearrange("p (s d) -> p s d", d=8)[:, :, 0:4],
                        in1=t1[:].rearrange("p (s d) -> p s d", d=8)[:, :, 4:8],
                        op=OP.add)
                    t3 = trp.tile([128, K * H * 2], bf16, tag="t3")
                    nc.vector.tensor_tensor(
                        out=t3[:].rearrange("p (s d) -> p s d", d=2),
                        in0=t2[:].rearrange("p (s d) -> p s d", d=4)[:, :, 0:2],
                        in1=t2[:].rearrange("p (s d) -> p s d", d=4)[:, :, 2:4],
                        op=OP.add)
                    sc = ssp.tile([128, K * H], bf16, tag="sc")
                    nc.vector.tensor_tensor(
                        out=sc[:].rearrange("p (s d) -> p s d", d=1),
                        in0=t3[:].rearrange("p (s d) -> p s d", d=2)[:, :, 0:1],
                        in1=t3[:].rearrange("p (s d) -> p s d", d=2)[:, :, 1:2],
                        op=OP.add)
                    # softmax (no max-subtract: |score| <= ~6, exp is safe)
                    ee = ssp.tile([128, K * H], bf16, tag="ee")
                    nc.scalar.activation(ee[:], sc[:], AF.Exp)
                    den = ssp.tile([128, H], f32, tag="den")
                    nc.vector.tensor_reduce(
                        out=den[:],
                        in_=ee[:].rearrange("p (k h) -> p h k", h=H),
                        axis=AX.X, op=OP.add)
                    rden = ssp.tile([128, H], f32, tag="rden")
                    nc.vector.reciprocal(rden[:], den[:])

                    # V aggregation: prod2[q,k,(hd,h)] = V_g * ee  (2x;
                    # V rows are hd-major so the ee broadcast is on the
                    # middle dim and the innermost stride stays 1)
                    prod2 = ppp.tile([128, K * D], bf16, tag="prod2")
                    nc.vector.tensor_tensor(
                        out=prod2[:].rearrange("p (k f h) -> p k f h", h=H, f=HD),
                        in0=kv3[:, :, D:E].rearrange("p k (f h) -> p k f h", h=H),
                        in1=ee[:].rearrange("p (k h) -> p k h", h=H)
                            .unsqueeze(2).broadcast_to([128, K, HD, H]),
                        op=OP.mult)
                    # halving-tree reduce over k: [q,(k,e)] -> att[q,e]
                    a1 = trp.tile([128, 16 * D], bf16, tag="a1")
                    nc.vector.tensor_tensor(
                        out=a1[:], in0=prod2[:, 0:16 * D],
                        in1=prod2[:, 16 * D:32 * D], op=OP.add)
                    a2 = trp.tile([128, 8 * D], bf16, tag="a2")
                    nc.vector.tensor_tensor(
                        out=a2[:], in0=a1[:, 0:8 * D], in1=a1[:, 8 * D:16 * D],
                        op=OP.add)
                    a3 = trp.tile([128, 4 * D], bf16, tag="a3")
                    nc.vector.tensor_tensor(
                        out=a3[:], in0=a2[:, 0:4 * D], in1=a2[:, 4 * D:8 * D],
                        op=OP.add)
                    a4 = trp.tile([128, 2 * D], bf16, tag="a4")
                    nc.vector.tensor_tensor(
                        out=a4[:], in0=a3[:, 0:2 * D], in1=a3[:, 2 * D:4 * D],
                        op=OP.add)
                    att = ssp.tile([128, D], f32, tag="att")
                    nc.vector.tensor_tensor(
                        out=att[:], in0=a4[:, 0:D], in1=a4[:, D:2 * D],
                        op=OP.add)
                    # normalize: attn[q,(hd,h)] = att * rden[q,h]
                    attn = ssp.tile([128, D], bf16, tag="attn")
                    nc.vector.tensor_tensor(
                        out=attn[:].rearrange("p (f h) -> p f h", h=H),
                        in0=att[:].rearrange("p (f h) -> p f h", h=H),
                        in1=rden[:].unsqueeze(1).broadcast_to([128, HD, H]),
                        op=OP.mult)

                    # output projection: out[q,:] = attn @ Wo_perm + bo_eff
                    psAT = psp.tile([128, 128], bf16, tag="psAT")
                    nc.tensor.transpose(psAT[:], attn[:], ident[:])
                    cAT = ssp.tile([128, 128], bf16, tag="cAT")
                    nc.scalar.activation(cAT[:], psAT[:], AF.Copy)
                    psO = psp.tile([128, D], f32, tag="psO")
                    nc.tensor.matmul(psO[:], lhsT=cAT[:], rhs=wo_s[:],
                                     start=True, stop=False)
                    nc.tensor.matmul(psO[:], lhsT=ones_s[:], rhs=bo_s[:],
                                     start=False, stop=True)
                    oT = ssp.tile([128, D], f32, tag="oT")
                    nc.scalar.activation(oT[:], psO[:], AF.Copy)
                    nc.sync.dma_start(outD[bass.ts(i, QT), :], oT[:])

    nc.compile()
    return nc


def _get_prog():
    global _PROG
    if _PROG is None:
        _PROG = _build()
    return _PROG


def _bf(x):
    return np.ascontiguousarray(np.asarray(x, np.float32)).astype(
        ml_dtypes.bfloat16)


def prep_in_maps(query_features, key_features, knn_indices,
                 Wq, bq, Wk, bk, Wv, bv, Wo, bo):
    """Host-side prep: shard queries, transpose/pad/cast, fold biases."""
    qf = np.asarray(query_features, np.float32)
    kf = np.asarray(key_features, np.float32)
    ki = np.asarray(knn_indices)

    # V hd-major permutation: col j=f*H+h of Wv_perm is col h*HD+f of Wv
    perm = np.arange(D).reshape(H, HD).T.reshape(-1)   # j -> h*HD+f
    Wv_p = np.asarray(Wv, np.float32)[:, perm]
    Wo_p = np.asarray(Wo, np.float32)[perm, :]
    wkv = np.concatenate(
        [np.asarray(Wk, np.float32), Wv_p], axis=1)    # [D, 2D]
    bo_eff = (np.asarray(bv, np.float32) @ np.asarray(Wo, np.float32)
              + np.asarray(bo, np.float32))            # bk drops out

    keysT = np.zeros((D, N2P), np.float32)
    keysT[:, :N2] = kf.T
    keysT = keysT.astype(ml_dtypes.bfloat16)
    wkv_b = _bf(wkv)
    wq_b = _bf(Wq)
    wo_b = _bf(Wo_p)
    bq_b = _bf(np.asarray(bq, np.float32).reshape(1, D))
    bo_b = _bf(bo_eff.reshape(1, D))

    in_maps = []
    for c in range(NCORES):
        qTc = np.zeros((D, N1P), np.float32)
        qTc[:, :N1C] = qf[c * N1C:(c + 1) * N1C].T
        knnc = np.zeros((N1P, K), np.int32)
        knnc[:N1C] = ki[c * N1C:(c + 1) * N1C].astype(np.int32)
        in_maps.append({
            "qT": qTc.astype(ml_dtypes.bfloat16), "keysT": keysT,
            "knn": knnc, "wq": wq_b, "wkv": wkv_b, "wo": wo_b,
            "bq": bq_b, "bo": bo_b,
        })
    return in_maps


def kernel(query_features, key_features, knn_indices,
           Wq, bq, Wk, bk, Wv, bv, Wo, bo):
    from concourse import bass_utils

    nc = _get_prog()
    in_maps = prep_in_maps(query_features, key_features, knn_indices,
                           Wq, bq, Wk, bk, Wv, bv, Wo, bo)
    res = bass_utils.run_bass_kernel_spmd(
        nc, in_maps, core_ids=list(range(NCORES)))

    out = np.empty((N1, D), np.float32)
    for c in range(NCORES):
        out[c * N1C:(c + 1) * N1C] = res.results[c]["outD"][:N1C, :]
    return out


# revision 9
# speedup vs baseline: 19.8489x; 19.8489x over previous
"""Local cross-attention (kNN gather) Trainium2 Bass kernel — v2.

Data-parallel over the 40000 query points across 8 NeuronCores.

Per core:
  Phase A: project all keys to a bf16 KV table in DRAM scratch
           (row n = [K-row(n) bf16 x128 | V-row(n) bf16 x128], 512B).
           Matmuls run key-major (lhsT = keysT chunk, rhs = [Wk|Wv_perm])
           so no PE transposes are needed; ACT copies PSUM->SBUF bf16.
  Phase B: per tile of 128 queries: 32 indirect-DMA row gathers (512B
           rows), scores + softmax + weighted sum on DVE in bf16 2x mode
           with halving-tree reductions, output projection on PE.

Bias algebra (exact):
  bk drops out (softmax is invariant to a per-(q,h) score shift).
  bv folds into the output bias: bo_eff = bv @ Wo + bo (host-side).
  bq is added into the Q projection via a ones-row accumulate matmul.

V layout: host permutes Wv columns hd-major (j = hd*H + h) so the
softmax-weight broadcast in the V aggregation has unit innermost stride
(keeps DVE in 2x mode); Wo rows are permuted to match.
"""

import numpy as np
import ml_dtypes

N1, N2, D, H, K = 40000, 60000, 128, 8, 32
HD = D // H
SCALE = HD ** -0.5
NCORES = 8
N1C = N1 // NCORES          # 5000 queries per core
QT = 128                    # queries per tile
N1P = 5120                  # padded queries per core -> 40 tiles
NT = N1P // QT
N2P = 60416                 # padded key count = 472*128
NSUB = N2P // 128           # phase-A subtiles of 128 keys
E = 2 * D                   # interleaved KV row length (256 bf16 = 512B)

_PROG = None


def _build():
    import concourse.bass as bass
    import concourse.tile as tile
    from concourse import bacc, mybir
    from contextlib import ExitStack

    f32 = mybir.dt.float32
    bf16 = mybir.dt.bfloat16
    i32 = mybir.dt.int32
    AX = mybir.AxisListType
    OP = mybir.AluOpType
    AF = mybir.ActivationFunctionType

    nc = bacc.Bacc("TRN2", target_bir_lowering=False, debug=False,
                   enable_asserts=False, num_devices=1)

    qT = nc.dram_tensor("qT", [D, N1P], bf16, kind="ExternalInput").ap()
    keysT = nc.dram_tensor("keysT", [D, N2P], bf16, kind="ExternalInput").ap()
    knn = nc.dram_tensor("knn", [N1P, K], i32, kind="ExternalInput").ap()
    wq = nc.dram_tensor("wq", [D, D], bf16, kind="ExternalInput").ap()
    wkv = nc.dram_tensor("wkv", [D, E], bf16, kind="ExternalInput").ap()
    wo = nc.dram_tensor("wo", [D, D], bf16, kind="ExternalInput").ap()
    bq = nc.dram_tensor("bq", [1, D], bf16, kind="ExternalInput").ap()
    bo = nc.dram_tensor("bo", [1, D], bf16, kind="ExternalInput").ap()
    outD = nc.dram_tensor("outD", [N1P, D], f32, kind="ExternalOutput").ap()
    table = nc.dram_tensor("kv_table", [N2P, E], bf16, kind="Internal").ap()

    with tile.TileContext(nc) as tc:
        with ExitStack() as cst:
            cp = cst.enter_context(tc.tile_pool(name="const", bufs=1))
            from concourse.masks import make_identity
            ident = cp.tile([128, 128], bf16)
            make_identity(nc, ident[:])
            wq_s = cp.tile([D, D], bf16, tag="wq")
            wkv_s = cp.tile([D, E], bf16, tag="wkv")
            wo_s = cp.tile([D, D], bf16, tag="wo")
            bq_s = cp.tile([1, D], bf16, tag="bq")
            bo_s = cp.tile([1, D], bf16, tag="bo")
            ones_s = cp.tile([1, QT], bf16, tag="ones")
            nc.vector.memset(ones_s[:], 1.0)
            for sb, dr in ((wq_s, wq), (wkv_s, wkv), (wo_s, wo),
                           (bq_s, bq), (bo_s, bo)):
                nc.sync.dma_start(sb[:], dr)
            qT_s = cp.tile([D, N1P], bf16, tag="qTs")
            nc.sync.dma_start(qT_s[:], qT)

            # ---- hoisted Q projections (independent of the KV table) ----
            qs_all = cp.tile([128, NT * D], bf16, tag="qs_all")
            with ExitStack() as qst:
                qps = qst.enter_context(
                    tc.tile_pool(name="q_ps", bufs=2, space="PSUM"))
                for i in range(NT):
                    psQ = qps.tile([128, D], f32, tag="psQ")
                    nc.tensor.matmul(psQ[:], lhsT=qT_s[:, bass.ts(i, QT)],
                                     rhs=wq_s[:], start=True, stop=False)
                    nc.tensor.matmul(psQ[:], lhsT=ones_s[:], rhs=bq_s[:],
                                     start=False, stop=True)
                    nc.scalar.activation(qs_all[:, bass.ts(i, D)], psQ[:],
                                         AF.Copy, scale=SCALE)

            # ---------------- Phase A: build KV table ----------------
            with ExitStack() as ast:
                ap_ = ast.enter_context(tc.tile_pool(name="pa_sb", bufs=3))
                kvp_ = ast.enter_context(tc.tile_pool(name="pa_kv", bufs=3))
                psA = ast.enter_context(
                    tc.tile_pool(name="pa_ps", bufs=4, space="PSUM"))
                for c in range(NSUB // 4):
                    kc = ap_.tile([128, 4 * 128], bf16, tag="kc")
                    nc.sync.dma_start(kc[:], keysT[:, bass.ts(c, 512)])
                    kvs = kvp_.tile([128, 4 * E], bf16, tag="kvs")
                    for t in range(4):
                        psKV = psA.tile([128, E], f32, tag="psKV")
                        nc.tensor.matmul(psKV[:],
                                         lhsT=kc[:, bass.ts(t, 128)],
                                         rhs=wkv_s[:], start=True, stop=True)
                        # split PSUM->SBUF copies across ACT and DVE
                        if t % 2 == 0:
                            nc.scalar.activation(kvs[:, bass.ts(t, E)],
                                                 psKV[:], AF.Copy)
                        else:
                            nc.vector.tensor_copy(kvs[:, bass.ts(t, E)],
                                                  psKV[:])
                    nc.sync.dma_start(
                        table[c * 512:(c + 1) * 512, :]
                        .rearrange("(t p) e -> p t e", p=128),
                        kvs[:].rearrange("p (t e) -> p t e", e=E))

            # Phase A writes kv_table in DRAM; Tile does not track DRAM
            # hazards, so fence before phase B gathers from it.
            tc.strict_bb_all_engine_barrier()

            # ---------------- Phase B: gather + attention ----------------
            with ExitStack() as bst:
                ixp = bst.enter_context(tc.tile_pool(name="pb_ix", bufs=3))
                kvp = bst.enter_context(tc.tile_pool(name="pb_kv", bufs=4))
                ppp = bst.enter_context(tc.tile_pool(name="pb_prod", bufs=2))
                trp = bst.enter_context(tc.tile_pool(name="pb_tree", bufs=2))
                ssp = bst.enter_context(tc.tile_pool(name="pb_small", bufs=3))
                psp = bst.enter_context(
                    tc.tile_pool(name="pb_ps", bufs=2, space="PSUM"))
                for i in range(NT):
                    idx = ixp.tile([128, K], i32, tag="idx")
                    nc.sync.dma_start(idx[:], knn[bass.ts(i, 128), :])
                    kv = kvp.tile([128, K * E], bf16, tag="kv")
                    kv3 = kv[:].rearrange("p (k e) -> p k e", e=E)
                    # one 512B row per partition per indirect DMA
                    for k in range(K):
                        nc.gpsimd.indirect_dma_start(
                            out=kv[:, k * E:(k + 1) * E],
                            out_offset=None, in_=table,
                            in_offset=bass.IndirectOffsetOnAxis(
                                ap=idx[:, k:k + 1], axis=0))

                    qs = qs_all[:, bass.ts(i, D)]

                    # scores: prod[q, k, d] = K_g[q,k,d] * qs[q,d]  (2x)
                    prod = ppp.tile([128, K * D], bf16, tag="prod")
                    nc.vector.tensor_tensor(
                        out=prod[:].rearrange("p (k d) -> p k d", d=D),
                        in0=kv3[:, :, 0:D],
                        in1=qs.unsqueeze(1).broadcast_to([128, K, D]),
                        op=OP.mult)
                    # halving-tree reduce over d within each head:
                    # prod layout (k, h, hd16) -> sc[q, (k,h)]
                    t1 = trp.tile([128, K * H * 8], bf16, tag="t1")
                    nc.vector.tensor_tensor(
                        out=t1[:].rearrange("p (s d) -> p s d", d=8),
                        in0=prod[:].rearrange("p (s d) -> p s d", d=16)[:, :, 0:8],
                        in1=prod[:].rearrange("p (s d) -> p s d", d=16)[:, :, 8:16],
                        op=OP.add)
                    t2 = trp.tile([128, K * H * 4], bf16, tag="t2")
                    nc.vector.tensor_tensor(
                        out=t2[:].rearrange("p (s d) -> p s d", d=4),
                        in0=t1[:].rearrange("p (s d) -> p s d", d=8)[:, :, 0:4],
                        in1=t1[:].rearrange("p (s d) -> p s d", d=8)[:, :, 4:8],
                        op=OP.add)
                    t3 = trp.tile([128, K * H * 2], bf16, tag="t3")
                    nc.vector.tensor_tensor(
                        out=t3[:].rearrange("p (s d) -> p s d", d=2),
                        in0=t2[:].rearrange("p (s d) -> p s d", d=4)[:, :, 0:2],
                        in1=t2[:].rearrange("p (s d) -> p s d", d=4)[:, :, 2:4],
                        op=OP.add)
                    sc = ssp.tile([128, K * H], bf16, tag="sc")
                    nc.vector.tensor_tensor(
                        out=sc[:].rearrange("p (s d) -> p s d", d=1),
                        in0=t3[:].rearrange("p (s d) -> p s d", d=2)[:, :, 0:1],
                        in1=t3[:].rearrange("p (s d) -> p s d", d=2)[:, :, 1:2],
                        op=OP.add)
                    # softmax (no max-subtract: |score| <= ~6, exp is safe)
                    ee = ssp.tile([128, K * H], bf16, tag="ee")
                    nc.scalar.activation(ee[:], sc[:], AF.Exp)
                    den = ssp.tile([128, H], f32, tag="den")
                    nc.vector.tensor_reduce(
                        out=den[:],
                        in_=ee[:].rearrange("p (k h) -> p h k", h=H),
                        axis=AX.X, op=OP.add)
                    rden = ssp.tile([128, H], f32, tag="rden")
                    nc.vector.reciprocal(rden[:], den[:])

                    # V aggregation: prod2[q,k,(hd,h)] = V_g * ee  (2x;
                    # V rows are hd-major so the ee broadcast is on the
                    # middle dim and the innermost stride stays 1)
                    prod2 = ppp.tile([128, K * D], bf16, tag="prod2")
                    nc.vector.tensor_tensor(
                        out=prod2[:].rearrange("p (k f h) -> p k f h", h=H, f=HD),
                        in0=kv3[:, :, D:E].rearrange("p k (f h) -> p k f h", h=H),
                        in1=ee[:].rearrange("p (k h) -> p k h", h=H)
                            .unsqueeze(2).broadcast_to([128, K, HD, H]),
                        op=OP.mult)
                    # halving-tree reduce over k: [q,(k,e)] -> att[q,e]
                    a1 = trp.tile([128, 16 * D], bf16, tag="a1")
                    nc.vector.tensor_tensor(
                        out=a1[:], in0=prod2[:, 0:16 * D],
                        in1=prod2[:, 16 * D:32 * D], op=OP.add)
                    a2 = trp.tile([128, 8 * D], bf16, tag="a2")
                    nc.vector.tensor_tensor(
                        out=a2[:], in0=a1[:, 0:8 * D], in1=a1[:, 8 * D:16 * D],
                        op=OP.add)
                    a3 = trp.tile([128, 4 * D], bf16, tag="a3")
                    nc.vector.tensor_tensor(
                        out=a3[:], in0=a2[:, 0:4 * D], in1=a2[:, 4 * D:8 * D],
                        op=OP.add)
                    a4 = trp.tile([128, 2 * D], bf16, tag="a4")
                    nc.vector.tensor_tensor(
                        out=a4[:], in0=a3[:, 0:2 * D], in1=a3[:, 2 * D:4 * D],
                        op=OP.add)
                    att = ssp.tile([128, D], f32, tag="att")
                    nc.vector.tensor_tensor(
                        out=att[:], in0=a4[:, 0:D], in1=a4[:, D:2 * D],
                        op=OP.add)
                    # normalize: attn[q,(hd,h)] = att * rden[q,h]
                    attn = ssp.tile([128, D], bf16, tag="attn")
                    nc.vector.tensor_tensor(
                        out=attn[:].rearrange("p (f h) -> p f h", h=H),
                        in0=att[:].rearrange("p (f h) -> p f h", h=H),
                        in1=rden[:].unsqueeze(1).broadcast_to([128, HD, H]),
                        op=OP.mult)

                    # output projection: out[q,:] = attn @ Wo_perm + bo_eff
                    psAT = psp.tile([128, 128], bf16, tag="psAT")
                    nc.tensor.transpose(psAT[:], attn[:], ident[:])
                    cAT = ssp.tile([128, 128], bf16, tag="cAT")
                    nc.scalar.activation(cAT[:], psAT[:], AF.Copy)
                    psO = psp.tile([128, D], f32, tag="psO")
                    nc.tensor.matmul(psO[:], lhsT=cAT[:], rhs=wo_s[:],
                                     start=True, stop=False)
                    nc.tensor.matmul(psO[:], lhsT=ones_s[:], rhs=bo_s[:],
                                     start=False, stop=True)
                    oT = ssp.tile([128, D], f32, tag="oT")
                    nc.scalar.activation(oT[:], psO[:], AF.Copy)
                    nc.sync.dma_start(outD[bass.ts(i, QT), :], oT[:])

    nc.compile()
    return nc


def _get_prog():
    global _PROG
    if _PROG is None:
        _PROG = _build()
    return _PROG


def _bf(x):
    return np.ascontiguousarray(np.asarray(x, np.float32)).astype(
        ml_dtypes.bfloat16)


def prep_in_maps(query_features, key_features, knn_indices,
                 Wq, bq, Wk, bk, Wv, bv, Wo, bo):
    """Host-side prep: shard queries, transpose/pad/cast, fold biases."""
    qf = np.asarray(query_features, np.float32)
    kf = np.asarray(key_features, np.float32)
    ki = np.asarray(knn_indices)

    # V hd-major permutation: col j=f*H+h of Wv_perm is col h*HD+f of Wv
    perm = np.arange(D).reshape(H, HD).T.reshape(-1)   # j -> h*HD+f
    Wv_p = np.asarray(Wv, np.float32)[:, perm]
    Wo_p = np.asarray(Wo, np.float32)[perm, :]
    wkv = np.concatenate(
        [np.asarray(Wk, np.float32), Wv_p], axis=1)    # [D, 2D]
    bo_eff = (np.asarray(bv, np.float32) @ np.asarray(Wo, np.float32)
              + np.asarray(bo, np.float32))            # bk drops out

    keysT = np.zeros((D, N2P), np.float32)
    keysT[:, :N2] = kf.T
    keysT = keysT.astype(ml_dtypes.bfloat16)
    wkv_b = _bf(wkv)
    wq_b = _bf(Wq)
    wo_b = _bf(Wo_p)
    bq_b = _bf(np.asarray(bq, np.float32).reshape(1, D))
    bo_b = _bf(bo_eff.reshape(1, D))

    in_maps = []
    for c in range(NCORES):
        qTc = np.zeros((D, N1P), np.float32)
        qTc[:, :N1C] = qf[c * N1C:(c + 1) * N1C].T
        knnc = np.zeros((N1P, K), np.int32)
        knnc[:N1C] = ki[c * N1C:(c + 1) * N1C].astype(np.int32)
        in_maps.append({
            "qT": qTc.astype(ml_dtypes.bfloat16), "keysT": keysT,
            "knn": knnc, "wq": wq_b, "wkv": wkv_b, "wo": wo_b,
            "bq": bq_b, "bo": bo_b,
        })
    return in_maps


def kernel(query_features, key_features, knn_indices,
           Wq, bq, Wk, bk, Wv, bv, Wo, bo):
    from concourse import bass_utils

    nc = _get_prog()
    in_maps = prep_in_maps(query_features, key_features, knn_indices,
                           Wq, bq, Wk, bk, Wv, bv, Wo, bo)
    res = bass_utils.run_bass_kernel_spmd(
        nc, in_maps, core_ids=list(range(NCORES)))

    out = np.empty((N1, D), np.float32)
    for c in range(NCORES):
        out[c * N1C:(c + 1) * N1C] = res.results[c]["outD"][:N1C, :]
    return out
